# revision 9
# baseline (speedup 1.0000x reference)
"""Child-Sum Tree-LSTM (T=1024 complete binary trees, L=256 leaves, in=300, mem=150)
on 8 Trainium2 NeuronCores via Bass/Tile.

Strategy:
- Data-parallel: 128 trees per core, no collectives.
- All on-chip state is transposed [feature, node-column]. Node columns are kept in
  bit-reversed order (applied host-side to x), so every level's child pair-sum is a
  contiguous first-half + second-half add.
- Matmuls run in float32r (full-rate fp32, ~1e-4 rounding) with K chunked to <=128.
- The 450 iou weight rows are permuted and zero-padded host-side to 470 rows
  [i128 | u128 | o128 | u22 0*10 i22 0*10 o22] so the remainder gates land at PSUM
  partitions 0/32/64 of one 86-row chunk (partition access must be 32-aligned), and
  every elementwise op sees base-partition-0 operands.
- Trees run in 16 blocks of 8; per block the tree reduces leaf->32 nodes/tree; block
  pairs merge for the 16-node level; the tail (8..1 nodes/tree) runs globally.
"""
import functools
import numpy as np

import concourse.bass as bass
import concourse.bacc as bacc
import concourse.tile as tile
from concourse import mybir
from concourse.bass_utils import run_bass_kernel_spmd

F32 = mybir.dt.float32
F32R = mybir.dt.float32r
AFT = mybir.ActivationFunctionType

T, L, IN, H = 1024, 256, 300, 150
GP = 470                # padded iou weight rows
NCORES = 8
TC = T // NCORES        # 128 trees per core
B = 8                   # trees per block
NB = TC // B            # 16 blocks
CB = B * L              # 2048 leaf columns per block
NT = 512                # column tile size

KX = [(0, 128), (128, 128), (256, 44)]   # K chunks of IN=300
KH = [(0, 128), (128, 22)]               # K chunks of H=150
MCH = [(0, 128), (128, 128), (256, 128), (384, 86)]  # M chunks of padded 470 rows
BREV = np.array([int(format(i, "08b")[::-1], 2) for i in range(L)])


def _pad470(w450):
    """[450, C] -> [470, C] permuted+padded: [i128|u128|o128|u22|0*10|i22|0*10|o22]."""
    w450 = np.asarray(w450, np.float32)
    out = np.zeros((GP,) + w450.shape[1:], np.float32)
    out[0:128] = w450[0:150][0:128]          # i big
    out[128:256] = w450[300:450][0:128]      # u big
    out[256:384] = w450[150:300][0:128]      # o big
    out[384:406] = w450[300:450][128:150]    # u rem @ 0:22 of chunk 3
    out[416:438] = w450[0:150][128:150]      # i rem @ 32:54
    out[448:470] = w450[150:300][128:150]    # o rem @ 64:86
    return out


def _gates_tail(nc, sb, pm, bt, fc_in, cb, cr, hb, hr, csl):
    """iou psum chunks -> gates -> c (=iu [+ fc]) -> tanh -> h. Writes state slices."""
    W = csl.stop - csl.start
    ib = sb.tile([128, NT], F32, tag="ib")
    ub = sb.tile([128, NT], F32, tag="ub")
    ob = sb.tile([128, NT], F32, tag="ob")
    ur = sb.tile([22, NT], F32, tag="ur", bufs=1)
    ir = sb.tile([22, NT], F32, tag="ir", bufs=1)
    orr = sb.tile([22, NT], F32, tag="orr", bufs=1)
    nc.scalar.activation(ib[:, :W], pm[0][0:128, :W], AFT.Sigmoid, bias=bt[:, 0:1])
    nc.scalar.activation(ub[:, :W], pm[1][0:128, :W], AFT.Tanh, bias=bt[:, 1:2])
    nc.scalar.activation(ob[:, :W], pm[2][0:128, :W], AFT.Sigmoid, bias=bt[:, 2:3])
    nc.scalar.activation(ur[:, :W], pm[3][0:22, :W], AFT.Tanh, bias=bt[0:22, 3:4])
    nc.scalar.activation(ir[:, :W], pm[3][32:54, :W], AFT.Sigmoid, bias=bt[0:22, 4:5])
    nc.scalar.activation(orr[:, :W], pm[3][64:86, :W], AFT.Sigmoid, bias=bt[0:22, 5:6])

    if fc_in is None:
        nc.vector.tensor_mul(cb[:, csl], ib[:, :W], ub[:, :W])
        nc.vector.tensor_mul(cr[:, csl], ir[:, :W], ur[:, :W])
    else:
        tmpb, tmpr = fc_in
        iub = sb.tile([128, NT], F32, tag="iub")
        iur = sb.tile([22, NT], F32, tag="iur", bufs=1)
        nc.vector.tensor_mul(iub[:, :W], ib[:, :W], ub[:, :W])
        nc.vector.tensor_mul(iur[:, :W], ir[:, :W], ur[:, :W])
        nc.vector.tensor_add(cb[:, csl], iub[:, :W], tmpb[:, :W])
        nc.vector.tensor_add(cr[:, csl], iur[:, :W], tmpr[:, :W])

    tcb = sb.tile([128, NT], F32, tag="tcb", bufs=1)
    tcr = sb.tile([22, NT], F32, tag="tcr", bufs=1)
    nc.scalar.activation(tcb[:, :W], cb[:, csl], AFT.Tanh)
    nc.scalar.activation(tcr[:, :W], cr[:, csl], AFT.Tanh)
    nc.vector.tensor_mul(hb[:, csl], ob[:, :W], tcb[:, :W])
    nc.vector.tensor_mul(hr[:, csl], orr[:, :W], tcr[:, :W])


def _leaf_level(nc, sb, ps, x_ap, b, wx_r, blt, ident, cb, cr, hb, hr):
    for ct in range(CB // NT):
        csl = slice(ct * NT, (ct + 1) * NT)
        xn = sb.tile([128, 4 * IN], F32, tag="xn", bufs=1)
        xn3 = xn[:].rearrange("p (g d) -> p g d", g=4)
        nc.sync.dma_start(
            xn3, x_ap[b, ct * NT:(ct + 1) * NT, :].rearrange("(g p) d -> p g d", p=128)
        )
        xts = []
        for k, (k0, kn) in enumerate(KX):
            pxt = ps.tile([128, NT], F32, tag=f"pxt{k}")
            for g in range(4):
                nc.tensor.transpose(
                    pxt[0:kn, g * 128:(g + 1) * 128], xn3[:, g, k0:k0 + kn], ident[:]
                )
            xt = sb.tile([128, NT], F32R, tag=f"xt{k}")
            nc.vector.tensor_copy(xt[0:kn, :], pxt[0:kn, :])
            xts.append(xt)
        pm = []
        for m, (m0, mw) in enumerate(MCH):
            pt = ps.tile([128, NT], F32, tag=f"pio{m}")
            for k, (k0, kn) in enumerate(KX):
                nc.tensor.matmul(
                    pt[0:mw, :],
                    wx_r[0:kn, k * GP + m0:k * GP + m0 + mw],
                    xts[k][0:kn, :],
                    start=(k == 0), stop=(k == len(KX) - 1),
                )
            pm.append(pt)
        _gates_tail(nc, sb, pm, blt, None, cb, cr, hb, hr, csl)


def _child_half(tile_ap, S, c0, W, half):
    """AP over child cols for parent range [c0, c0+W); returns (ap, is_multiseg)."""
    if W <= S:
        b, s0 = divmod(c0, S)
        base = b * 2 * S + half * S + s0
        return tile_ap[:, base:base + W], False
    assert c0 % S == 0 and W % S == 0
    v = tile_ap.rearrange("p (nb two s) -> p nb two s", two=2, s=S)
    return v[:, c0 // S:(c0 + W) // S, half, :], True


def _internal_level(nc, sb, ps, P, nseg, wh_r, wf_r, bit, bft, child, parent):
    """One tree level: P parent cols from 2P child cols (nseg segments)."""
    ccb, ccr, chb, chr_ = child
    cb, cr, hb, hr = parent
    S = P // nseg
    for ct in range((P + NT - 1) // NT):
        c0 = ct * NT
        W = min(NT, P - c0)
        csl = slice(c0, c0 + W)

        def seg(ap, tag_w=W):
            return ap.rearrange("p (nb s) -> p nb s", s=S) if S < tag_w else ap

        # --- child-sum of h ---
        hsb = sb.tile([128, NT], F32R, tag="hsb", bufs=1)
        hsr = sb.tile([22, NT], F32R, tag="hsr", bufs=1)
        hL, mseg = _child_half(chb[:], S, c0, W, 0)
        hR, _ = _child_half(chb[:], S, c0, W, 1)
        nc.vector.tensor_add(seg(hsb[:, :W]) if mseg else hsb[:, :W],
                             hL.bitcast(F32), hR.bitcast(F32))
        rL, _ = _child_half(chr_[:], S, c0, W, 0)
        rR, _ = _child_half(chr_[:], S, c0, W, 1)
        nc.vector.tensor_add(seg(hsr[:, :W]) if mseg else hsr[:, :W],
                             rL.bitcast(F32), rR.bitcast(F32))
        # --- iou matmuls ---
        pm = []
        for m, (m0, mw) in enumerate(MCH):
            pt = ps.tile([128, NT], F32, tag=f"pio{m}")
            for k, (k0, kn) in enumerate(KH):
                rhs = hsb[0:kn, :W] if k == 0 else hsr[0:22, :W]
                nc.tensor.matmul(
                    pt[0:mw, :W], wh_r[0:kn, k * GP + m0:k * GP + m0 + mw], rhs,
                    start=(k == 0), stop=(k == 1),
                )
            pm.append(pt)
        # --- forget gates per child half + fc ---
        fcbs, fcrs = [], []
        for half in range(2):
            hc, mh = _child_half(chb[:], S, c0, W, half)
            hcr, _ = _child_half(chr_[:], S, c0, W, half)
            pfb = ps.tile([128, NT], F32, tag=f"pxt{half}")
            pfr = ps.tile([22, NT], F32, tag=("pxt2" if half == 0 else "pfr1"),
                          name=f"pfr{half}")
            for k, (k0, kn) in enumerate(KH):
                rhs = hc if k == 0 else hcr
                nc.tensor.matmul(
                    pfb[0:128, :W], wf_r[0:kn, k * H + 0:k * H + 128], rhs,
                    start=(k == 0), stop=(k == 1), tile_position=(0, 0),
                )
                nc.tensor.matmul(
                    pfr[0:22, :W],
                    wf_r[0:kn, k * H + 128:k * H + 150], rhs,
                    start=(k == 0), stop=(k == 1), tile_position=(0, 0),
                )
            fb = sb.tile([128, NT], F32, tag=f"fb{half}")
            nc.scalar.activation(fb[:, :W], pfb[0:128, :W], AFT.Sigmoid,
                                 bias=bft[:, 0:1])
            fr = sb.tile([22, NT], F32, tag=f"fr{half}", bufs=1)
            nc.scalar.activation(fr[:, :W], pfr[0:22, :W],
                                 AFT.Sigmoid, bias=bft[0:22, 1:2])
            ccL, _ = _child_half(ccb[:], S, c0, W, half)
            crL, _ = _child_half(ccr[:], S, c0, W, half)
            fcb = sb.tile([128, NT], F32, tag=f"fcb{half}")
            fcr = sb.tile([22, NT], F32, tag=f"fcr{half}", bufs=1)
            nc.vector.tensor_mul(seg(fcb[:, :W]) if mh else fcb[:, :W],
                                 seg(fb[:, :W]) if mh else fb[:, :W], ccL)
            nc.vector.tensor_mul(seg(fcr[:, :W]) if mh else fcr[:, :W],
                                 seg(fr[:, :W]) if mh else fr[:, :W], crL)
            fcbs.append(fcb)
            fcrs.append(fcr)
        tmpb = sb.tile([128, NT], F32, tag="tmpb")
        tmpr = sb.tile([22, NT], F32, tag="tmpr", bufs=1)
        nc.vector.tensor_add(tmpb[:, :W], fcbs[0][:, :W], fcbs[1][:, :W])
        nc.vector.tensor_add(tmpr[:, :W], fcrs[0][:, :W], fcrs[1][:, :W])
        _gates_tail(nc, sb, pm, bit, (tmpb, tmpr), cb, cr, hb, hr, csl)


@functools.cache
def _build():
    nc = bacc.Bacc("TRN2", target_bir_lowering=False, debug=False,
                   num_devices=NCORES)
    x = nc.dram_tensor("x", [NB, CB, IN], F32, kind="ExternalInput").ap()
    wx = nc.dram_tensor("wx", [IN, GP], F32, kind="ExternalInput").ap()
    wh = nc.dram_tensor("wh", [H, GP], F32, kind="ExternalInput").ap()
    wf = nc.dram_tensor("wf", [H, H], F32, kind="ExternalInput").ap()
    bl = nc.dram_tensor("bl", [128, 6], F32, kind="ExternalInput").ap()
    bi = nc.dram_tensor("bi", [128, 6], F32, kind="ExternalInput").ap()
    bf = nc.dram_tensor("bf", [128, 2], F32, kind="ExternalInput").ap()
    idd = nc.dram_tensor("idd", [128, 128], F32, kind="ExternalInput").ap()
    out = nc.dram_tensor("out", [2, TC, H], F32, kind="ExternalOutput").ap()

    with tile.TileContext(nc, trace_sim=False) as tc:
        with (
            tc.tile_pool(name="const", bufs=1) as cpool,
            tc.tile_pool(name="state", bufs=1) as stp,
            tc.tile_pool(name="sb", bufs=2) as sb,
            tc.tile_pool(name="ps", bufs=1, space="PSUM") as ps,
        ):
            ident = cpool.tile([128, 128], F32, tag="ident")
            nc.sync.dma_start(ident[:], idd[:])
            blt = cpool.tile([128, 6], F32, tag="blt")
            nc.sync.dma_start(blt[:], bl[:])
            bit = cpool.tile([128, 6], F32, tag="bit")
            nc.sync.dma_start(bit[:], bi[:])
            bft = cpool.tile([128, 2], F32, tag="bft")
            nc.sync.dma_start(bft[:], bf[:])

            wx_r = cpool.tile([128, 3 * GP], F32R, tag="wx_r")
            wh_r = cpool.tile([128, 2 * GP], F32R, tag="wh_r")
            wf_r = cpool.tile([128, 2 * H], F32R, tag="wf_r")
            stg = cpool.tile([128, 3 * GP], F32, tag="stg")
            for k, (k0, kn) in enumerate(KX):
                nc.sync.dma_start(stg[0:kn, k * GP:(k + 1) * GP], wx[k0:k0 + kn, :])
                nc.vector.tensor_copy(wx_r[0:kn, k * GP:(k + 1) * GP],
                                      stg[0:kn, k * GP:(k + 1) * GP])
            for k, (k0, kn) in enumerate(KH):
                nc.sync.dma_start(stg[0:kn, k * GP:k * GP + GP], wh[k0:k0 + kn, :])
                nc.vector.tensor_copy(wh_r[0:kn, k * GP:(k + 1) * GP],
                                      stg[0:kn, k * GP:k * GP + GP])
                nc.sync.dma_start(stg[0:kn, 2 * GP + k * H:2 * GP + (k + 1) * H],
                                  wf[k0:k0 + kn, :])
                nc.vector.tensor_copy(wf_r[0:kn, k * H:(k + 1) * H],
                                      stg[0:kn, 2 * GP + k * H:2 * GP + (k + 1) * H])

            def state(cols, tag):
                return (
                    stp.tile([128, cols], F32, tag=f"cb_{tag}", name=f"cb_{tag}"),
                    stp.tile([22, cols], F32, tag=f"cr_{tag}", name=f"cr_{tag}"),
                    stp.tile([128, cols], F32R, tag=f"hb_{tag}", name=f"hb_{tag}"),
                    stp.tile([22, cols], F32R, tag=f"hr_{tag}", name=f"hr_{tag}"),
                )

            g16 = state(TC * 16, "G")
            for bp in range(NB // 2):
                g32 = state(512, "P")
                for half, b in enumerate((2 * bp, 2 * bp + 1)):
                    stA = state(CB, "A")
                    _leaf_level(nc, sb, ps, x, b, wx_r, blt, ident, *stA)
                    stB = state(CB // 2, "B")
                    _internal_level(nc, sb, ps, CB // 2, 1, wh_r, wf_r, bit, bft,
                                    stA, stB)
                    stA2 = state(CB // 4, "A")
                    _internal_level(nc, sb, ps, CB // 4, 1, wh_r, wf_r, bit, bft,
                                    stB, stA2)
                    g32h = tuple(t[:, half * 256:(half + 1) * 256] for t in g32)
                    _internal_level(nc, sb, ps, CB // 8, 1, wh_r, wf_r, bit, bft,
                                    stA2, g32h)
                g16s = tuple(t[:, bp * 256:(bp + 1) * 256] for t in g16)
                _internal_level(nc, sb, ps, 256, 2, wh_r, wf_r, bit, bft, g32, g16s)

            cur = g16
            for P in (TC * 8, TC * 4, TC * 2, TC):
                tag = "A" if P in (TC * 8, TC * 2) else "B"
                nxt = state(P, tag)
                _internal_level(nc, sb, ps, P, 16, wh_r, wf_r, bit, bft, cur, nxt)
                cur = nxt

            # --- output: transpose root c,h to [tree, feature] and DMA out ---
            cbr, crr, hbr, hrr = cur
            for idx, (big, rm) in enumerate(((cbr, crr), (hbr, hrr))):
                po = ps.tile([128, 256], F32, tag=f"pio{idx}")
                bg = big[:] if idx == 0 else big[:].bitcast(F32)
                rg = rm[:] if idx == 0 else rm[:].bitcast(F32)
                nc.tensor.transpose(po[0:128, 0:128], bg, ident[:])
                nc.tensor.transpose(po[0:128, 128:150], rg, ident[0:22, 0:22],
                                    tile_position=(0, 0))
                nat = sb.tile([128, H], F32, tag=f"nat{idx}", bufs=1)
                nc.vector.tensor_copy(nat[:], po[:, 0:H])
                nc.sync.dma_start(out[idx], nat[:])

    nc.compile()
    return nc


def _pack_bias450(v):
    vp = _pad470(np.asarray(v, np.float32).reshape(450, 1))[:, 0]
    arr = np.zeros((128, 6), np.float32)
    arr[:, 0], arr[:, 1], arr[:, 2] = vp[0:128], vp[128:256], vp[256:384]
    arr[0:22, 3] = vp[384:406]   # u rem
    arr[0:22, 4] = vp[416:438]   # i rem
    arr[0:22, 5] = vp[448:470]   # o rem
    return arr


def _pack_bias150(v):
    v = np.asarray(v, np.float32)
    arr = np.zeros((128, 2), np.float32)
    arr[:, 0] = v[0:128]
    arr[0:22, 1] = v[128:150]
    return arr


# test-harness hooks: set _RUN_KWARGS["trace"]=True before calling kernel() to
# capture an NTFF profile; the BassKernelResults lands in _LAST[0].
_RUN_KWARGS = {}
_LAST = [None]


def kernel(x, W_ioux, b_ioux, W_iouh, b_iouh, W_fx, b_fx, W_fh, b_fh):
    # W_fx/b_fx only affect leaf forget gates, which multiply zero child cells
    # in the reference; they do not influence the output.
    nc = _build()
    xp = np.asarray(x, np.float32)[:, BREV, :]
    consts = {
        "wx": np.ascontiguousarray(_pad470(np.asarray(W_ioux, np.float32)).T),
        "wh": np.ascontiguousarray(_pad470(np.asarray(W_iouh, np.float32)).T),
        "wf": np.ascontiguousarray(np.asarray(W_fh, np.float32).T),
        "bl": _pack_bias450(np.asarray(b_ioux) + np.asarray(b_iouh)),
        "bi": _pack_bias450(b_iouh),
        "bf": _pack_bias150(b_fh),
        "idd": np.eye(128, dtype=np.float32),
    }
    in_maps = []
    for c in range(NCORES):
        xc = xp[c * TC:(c + 1) * TC]                          # [128, 256, 300]
        xb = xc.reshape(NB, B, L, IN).transpose(0, 2, 1, 3)   # [16, 256, 8, 300]
        in_maps.append({"x": np.ascontiguousarray(xb.reshape(NB, CB, IN)), **consts})
    res = run_bass_kernel_spmd(nc, in_maps, core_ids=list(range(NCORES)),
                               **_RUN_KWARGS)
    _LAST[0] = res
    return np.concatenate([res.results[c]["out"] for c in range(NCORES)], axis=1)


# revision 11
# speedup vs baseline: 1.0345x; 1.0345x over previous
"""Child-Sum Tree-LSTM (T=1024 complete binary trees, L=256 leaves, in=300, mem=150)
on 8 Trainium2 NeuronCores via Bass/Tile.

Strategy:
- Data-parallel: 128 trees per core, no collectives.
- All on-chip state is transposed [feature, node-column]. Node columns are kept in
  bit-reversed order (applied host-side to x), so every level's child pair-sum is a
  contiguous first-half + second-half add.
- Matmuls run in float32r (full-rate fp32, ~1e-4 rounding) with K chunked to <=128.
- The 450 iou weight rows are permuted and zero-padded host-side to 470 rows
  [i128 | u128 | o128 | u22 0*10 i22 0*10 o22] so the remainder gates land at PSUM
  partitions 0/32/64 of one 86-row chunk (partition access must be 32-aligned), and
  every elementwise op sees base-partition-0 operands.
- Trees run in 16 blocks of 8; per block the tree reduces leaf->32 nodes/tree; block
  pairs merge for the 16-node level; the tail (8..1 nodes/tree) runs globally.
"""
import functools
import numpy as np

import concourse.bass as bass
import concourse.bacc as bacc
import concourse.tile as tile
from concourse import mybir
from concourse.bass_utils import run_bass_kernel_spmd

F32 = mybir.dt.float32
F32R = mybir.dt.float32r
BF16 = mybir.dt.bfloat16
AFT = mybir.ActivationFunctionType

T, L, IN, H = 1024, 256, 300, 150
GP = 470                # padded iou weight rows
NCORES = 8
TC = T // NCORES        # 128 trees per core
B = 8                   # trees per block
NB = TC // B            # 16 blocks
CB = B * L              # 2048 leaf columns per block
NT = 512                # column tile size

KX = [(0, 128), (128, 128), (256, 44)]   # K chunks of IN=300
KH = [(0, 128), (128, 22)]               # K chunks of H=150
MCH = [(0, 128), (128, 128), (256, 128), (384, 86)]  # M chunks of padded 470 rows
BREV = np.array([int(format(i, "08b")[::-1], 2) for i in range(L)])


def _pad470(w450):
    """[450, C] -> [470, C] permuted+padded: [i128|u128|o128|u22|0*10|i22|0*10|o22]."""
    w450 = np.asarray(w450, np.float32)
    out = np.zeros((GP,) + w450.shape[1:], np.float32)
    out[0:128] = w450[0:150][0:128]          # i big
    out[128:256] = w450[300:450][0:128]      # u big
    out[256:384] = w450[150:300][0:128]      # o big
    out[384:406] = w450[300:450][128:150]    # u rem @ 0:22 of chunk 3
    out[416:438] = w450[0:150][128:150]      # i rem @ 32:54
    out[448:470] = w450[150:300][128:150]    # o rem @ 64:86
    return out


def _gates_tail(nc, sb, pm, bt, fc_in, cb, cr, hb, hr, csl):
    """iou psum chunks -> gates -> c (=iu [+ fc]) -> tanh -> h. Writes state slices."""
    W = csl.stop - csl.start
    ib = sb.tile([128, NT], BF16, tag="ib")
    ub = sb.tile([128, NT], BF16, tag="ub")
    ob = sb.tile([128, NT], BF16, tag="ob")
    ur = sb.tile([22, NT], BF16, tag="ur", bufs=1)
    ir = sb.tile([22, NT], BF16, tag="ir", bufs=1)
    orr = sb.tile([22, NT], BF16, tag="orr", bufs=1)
    nc.scalar.activation(ib[:, :W], pm[0][0:128, :W], AFT.Sigmoid, bias=bt[:, 0:1])
    nc.scalar.activation(ub[:, :W], pm[1][0:128, :W], AFT.Tanh, bias=bt[:, 1:2])
    nc.scalar.activation(ob[:, :W], pm[2][0:128, :W], AFT.Sigmoid, bias=bt[:, 2:3])
    nc.scalar.activation(ur[:, :W], pm[3][0:22, :W], AFT.Tanh, bias=bt[0:22, 3:4])
    nc.scalar.activation(ir[:, :W], pm[3][32:54, :W], AFT.Sigmoid, bias=bt[0:22, 4:5])
    nc.scalar.activation(orr[:, :W], pm[3][64:86, :W], AFT.Sigmoid, bias=bt[0:22, 5:6])

    if fc_in is None:
        nc.vector.tensor_mul(cb[:, csl], ib[:, :W], ub[:, :W])
        nc.vector.tensor_mul(cr[:, csl], ir[:, :W], ur[:, :W])
    else:
        tmpb, tmpr = fc_in
        iub = sb.tile([128, NT], BF16, tag="iub")
        iur = sb.tile([22, NT], BF16, tag="iur", bufs=1)
        nc.vector.tensor_mul(iub[:, :W], ib[:, :W], ub[:, :W])
        nc.vector.tensor_mul(iur[:, :W], ir[:, :W], ur[:, :W])
        nc.vector.tensor_add(cb[:, csl], iub[:, :W], tmpb[:, :W])
        nc.vector.tensor_add(cr[:, csl], iur[:, :W], tmpr[:, :W])

    tcb = sb.tile([128, NT], BF16, tag="tcb", bufs=1)
    tcr = sb.tile([22, NT], BF16, tag="tcr", bufs=1)
    nc.scalar.activation(tcb[:, :W], cb[:, csl], AFT.Tanh)
    nc.scalar.activation(tcr[:, :W], cr[:, csl], AFT.Tanh)
    nc.vector.tensor_mul(hb[:, csl], ob[:, :W], tcb[:, :W])
    nc.vector.tensor_mul(hr[:, csl], orr[:, :W], tcr[:, :W])


def _leaf_level(nc, sb, ps, x_ap, b, wx_r, blt, ident, cb, cr, hb, hr):
    for ct in range(CB // NT):
        csl = slice(ct * NT, (ct + 1) * NT)
        xn = sb.tile([128, 4 * IN], F32, tag="xn", bufs=1)
        xn3 = xn[:].rearrange("p (g d) -> p g d", g=4)
        nc.sync.dma_start(
            xn3, x_ap[b, ct * NT:(ct + 1) * NT, :].rearrange("(g p) d -> p g d", p=128)
        )
        xnb = sb.tile([128, 4 * IN], BF16, tag="xnb", bufs=1)
        nc.gpsimd.tensor_copy(xnb[:], xn[:])
        xnb3 = xnb[:].rearrange("p (g d) -> p g d", g=4)
        xts = []
        for k, (k0, kn) in enumerate(KX):
            pxt = ps.tile([128, NT], BF16, tag=f"pxt{k}")
            for g in range(4):
                nc.tensor.transpose(
                    pxt[0:kn, g * 128:(g + 1) * 128], xnb3[:, g, k0:k0 + kn], ident[:]
                )
            xt = sb.tile([128, NT], BF16, tag=f"xt{k}")
            nc.vector.tensor_copy(xt[0:kn, :], pxt[0:kn, :])
            xts.append(xt)
        pm = []
        for m, (m0, mw) in enumerate(MCH):
            pt = ps.tile([128, NT], F32, tag=f"pio{m}")
            for k, (k0, kn) in enumerate(KX):
                nc.tensor.matmul(
                    pt[0:mw, :],
                    wx_r[0:kn, k * GP + m0:k * GP + m0 + mw],
                    xts[k][0:kn, :],
                    start=(k == 0), stop=(k == len(KX) - 1),
                )
            pm.append(pt)
        _gates_tail(nc, sb, pm, blt, None, cb, cr, hb, hr, csl)


def _child_half(tile_ap, S, c0, W, half):
    """AP over child cols for parent range [c0, c0+W); returns (ap, is_multiseg)."""
    if W <= S:
        b, s0 = divmod(c0, S)
        base = b * 2 * S + half * S + s0
        return tile_ap[:, base:base + W], False
    assert c0 % S == 0 and W % S == 0
    v = tile_ap.rearrange("p (nb two s) -> p nb two s", two=2, s=S)
    return v[:, c0 // S:(c0 + W) // S, half, :], True


def _internal_level(nc, sb, ps, P, nseg, wh_r, wf_r, bit, bft, child, parent):
    """One tree level: P parent cols from 2P child cols (nseg segments)."""
    ccb, ccr, chb, chr_ = child
    cb, cr, hb, hr = parent
    S = P // nseg
    for ct in range((P + NT - 1) // NT):
        c0 = ct * NT
        W = min(NT, P - c0)
        csl = slice(c0, c0 + W)

        def seg(ap, tag_w=W):
            return ap.rearrange("p (nb s) -> p nb s", s=S) if S < tag_w else ap

        # --- child-sum of h ---
        hsb = sb.tile([128, NT], BF16, tag="hsb", bufs=1)
        hsr = sb.tile([22, NT], BF16, tag="hsr", bufs=1)
        hL, mseg = _child_half(chb[:], S, c0, W, 0)
        hR, _ = _child_half(chb[:], S, c0, W, 1)
        nc.vector.tensor_add(seg(hsb[:, :W]) if mseg else hsb[:, :W],
                             hL, hR)
        rL, _ = _child_half(chr_[:], S, c0, W, 0)
        rR, _ = _child_half(chr_[:], S, c0, W, 1)
        nc.vector.tensor_add(seg(hsr[:, :W]) if mseg else hsr[:, :W],
                             rL, rR)
        # --- iou matmuls ---
        pm = []
        for m, (m0, mw) in enumerate(MCH):
            pt = ps.tile([128, NT], F32, tag=f"pio{m}")
            for k, (k0, kn) in enumerate(KH):
                rhs = hsb[0:kn, :W] if k == 0 else hsr[0:22, :W]
                nc.tensor.matmul(
                    pt[0:mw, :W], wh_r[0:kn, k * GP + m0:k * GP + m0 + mw], rhs,
                    start=(k == 0), stop=(k == 1),
                )
            pm.append(pt)
        # --- forget gates per child half + fc ---
        fcbs, fcrs = [], []
        for half in range(2):
            hc, mh = _child_half(chb[:], S, c0, W, half)
            hcr, _ = _child_half(chr_[:], S, c0, W, half)
            pfb = ps.tile([128, NT], F32, tag=f"pxt{half}")
            pfr = ps.tile([22, NT], F32, tag=("pxt2" if half == 0 else "pfr1"),
                          name=f"pfr{half}")
            for k, (k0, kn) in enumerate(KH):
                rhs = hc if k == 0 else hcr
                nc.tensor.matmul(
                    pfb[0:128, :W], wf_r[0:kn, k * H + 0:k * H + 128], rhs,
                    start=(k == 0), stop=(k == 1), tile_position=(0, 0),
                )
                nc.tensor.matmul(
                    pfr[0:22, :W],
                    wf_r[0:kn, k * H + 128:k * H + 150], rhs,
                    start=(k == 0), stop=(k == 1), tile_position=(0, 0),
                )
            fb = sb.tile([128, NT], BF16, tag=f"fb{half}")
            nc.scalar.activation(fb[:, :W], pfb[0:128, :W], AFT.Sigmoid,
                                 bias=bft[:, 0:1])
            fr = sb.tile([22, NT], BF16, tag=f"fr{half}", bufs=1)
            nc.scalar.activation(fr[:, :W], pfr[0:22, :W],
                                 AFT.Sigmoid, bias=bft[0:22, 1:2])
            ccL, _ = _child_half(ccb[:], S, c0, W, half)
            crL, _ = _child_half(ccr[:], S, c0, W, half)
            fcb = sb.tile([128, NT], BF16, tag=f"fcb{half}")
            fcr = sb.tile([22, NT], BF16, tag=f"fcr{half}", bufs=1)
            nc.vector.tensor_mul(seg(fcb[:, :W]) if mh else fcb[:, :W],
                                 seg(fb[:, :W]) if mh else fb[:, :W], ccL)
            nc.vector.tensor_mul(seg(fcr[:, :W]) if mh else fcr[:, :W],
                                 seg(fr[:, :W]) if mh else fr[:, :W], crL)
            fcbs.append(fcb)
            fcrs.append(fcr)
        tmpb = sb.tile([128, NT], BF16, tag="tmpb")
        tmpr = sb.tile([22, NT], BF16, tag="tmpr", bufs=1)
        nc.vector.tensor_add(tmpb[:, :W], fcbs[0][:, :W], fcbs[1][:, :W])
        nc.vector.tensor_add(tmpr[:, :W], fcrs[0][:, :W], fcrs[1][:, :W])
        _gates_tail(nc, sb, pm, bit, (tmpb, tmpr), cb, cr, hb, hr, csl)


@functools.cache
def _build():
    nc = bacc.Bacc("TRN2", target_bir_lowering=False, debug=False,
                   num_devices=NCORES)
    x = nc.dram_tensor("x", [NB, CB, IN], F32, kind="ExternalInput").ap()
    wx = nc.dram_tensor("wx", [IN, GP], F32, kind="ExternalInput").ap()
    wh = nc.dram_tensor("wh", [H, GP], F32, kind="ExternalInput").ap()
    wf = nc.dram_tensor("wf", [H, H], F32, kind="ExternalInput").ap()
    bl = nc.dram_tensor("bl", [128, 6], F32, kind="ExternalInput").ap()
    bi = nc.dram_tensor("bi", [128, 6], F32, kind="ExternalInput").ap()
    bf = nc.dram_tensor("bf", [128, 2], F32, kind="ExternalInput").ap()
    idd = nc.dram_tensor("idd", [128, 128], F32, kind="ExternalInput").ap()
    out = nc.dram_tensor("out", [2, TC, H], F32, kind="ExternalOutput").ap()

    with tile.TileContext(nc, trace_sim=False) as tc:
        with (
            tc.tile_pool(name="const", bufs=1) as cpool,
            tc.tile_pool(name="state", bufs=1) as stp,
            tc.tile_pool(name="sb", bufs=2) as sb,
            tc.tile_pool(name="ps", bufs=1, space="PSUM") as ps,
        ):
            identf = cpool.tile([128, 128], F32, tag="identf")
            nc.sync.dma_start(identf[:], idd[:])
            ident = cpool.tile([128, 128], BF16, tag="ident")
            nc.vector.tensor_copy(ident[:], identf[:])
            blt = cpool.tile([128, 6], F32, tag="blt")
            nc.sync.dma_start(blt[:], bl[:])
            bit = cpool.tile([128, 6], F32, tag="bit")
            nc.sync.dma_start(bit[:], bi[:])
            bft = cpool.tile([128, 2], F32, tag="bft")
            nc.sync.dma_start(bft[:], bf[:])

            wx_r = cpool.tile([128, 3 * GP], BF16, tag="wx_r")
            wh_r = cpool.tile([128, 2 * GP], BF16, tag="wh_r")
            wf_r = cpool.tile([128, 2 * H], BF16, tag="wf_r")
            stg = cpool.tile([128, 3 * GP], F32, tag="stg")
            for k, (k0, kn) in enumerate(KX):
                nc.sync.dma_start(stg[0:kn, k * GP:(k + 1) * GP], wx[k0:k0 + kn, :])
                nc.vector.tensor_copy(wx_r[0:kn, k * GP:(k + 1) * GP],
                                      stg[0:kn, k * GP:(k + 1) * GP])
            for k, (k0, kn) in enumerate(KH):
                nc.sync.dma_start(stg[0:kn, k * GP:k * GP + GP], wh[k0:k0 + kn, :])
                nc.vector.tensor_copy(wh_r[0:kn, k * GP:(k + 1) * GP],
                                      stg[0:kn, k * GP:k * GP + GP])
                nc.sync.dma_start(stg[0:kn, 2 * GP + k * H:2 * GP + (k + 1) * H],
                                  wf[k0:k0 + kn, :])
                nc.vector.tensor_copy(wf_r[0:kn, k * H:(k + 1) * H],
                                      stg[0:kn, 2 * GP + k * H:2 * GP + (k + 1) * H])

            def state(cols, tag):
                return (
                    stp.tile([128, cols], BF16, tag=f"cb_{tag}", name=f"cb_{tag}"),
                    stp.tile([22, cols], BF16, tag=f"cr_{tag}", name=f"cr_{tag}"),
                    stp.tile([128, cols], BF16, tag=f"hb_{tag}", name=f"hb_{tag}"),
                    stp.tile([22, cols], BF16, tag=f"hr_{tag}", name=f"hr_{tag}"),
                )

            g16 = state(TC * 16, "G")
            for bp in range(NB // 2):
                g32 = state(512, "P")
                for half, b in enumerate((2 * bp, 2 * bp + 1)):
                    stA = state(CB, "A")
                    _leaf_level(nc, sb, ps, x, b, wx_r, blt, ident, *stA)
                    stB = state(CB // 2, "B")
                    _internal_level(nc, sb, ps, CB // 2, 1, wh_r, wf_r, bit, bft,
                                    stA, stB)
                    stA2 = state(CB // 4, "A")
                    _internal_level(nc, sb, ps, CB // 4, 1, wh_r, wf_r, bit, bft,
                                    stB, stA2)
                    g32h = tuple(t[:, half * 256:(half + 1) * 256] for t in g32)
                    _internal_level(nc, sb, ps, CB // 8, 1, wh_r, wf_r, bit, bft,
                                    stA2, g32h)
                g16s = tuple(t[:, bp * 256:(bp + 1) * 256] for t in g16)
                _internal_level(nc, sb, ps, 256, 2, wh_r, wf_r, bit, bft, g32, g16s)

            cur = g16
            for P in (TC * 8, TC * 4, TC * 2, TC):
                tag = "A" if P in (TC * 8, TC * 2) else "B"
                nxt = state(P, tag)
                _internal_level(nc, sb, ps, P, 16, wh_r, wf_r, bit, bft, cur, nxt)
                cur = nxt

            # --- output: transpose root c,h to [tree, feature] and DMA out ---
            cbr, crr, hbr, hrr = cur
            for idx, (big, rm) in enumerate(((cbr, crr), (hbr, hrr))):
                po = ps.tile([128, 256], BF16, tag=f"pxt{idx}", name=f"po{idx}")
                bg = big[:]
                rg = rm[:]
                nc.tensor.transpose(po[0:128, 0:128], bg, ident[:])
                nc.tensor.transpose(po[0:128, 128:150], rg, ident[0:22, 0:22],
                                    tile_position=(0, 0))
                nat = sb.tile([128, H], F32, tag=f"nat{idx}", bufs=1)
                nc.vector.tensor_copy(nat[:], po[:, 0:H])
                nc.sync.dma_start(out[idx], nat[:])

    nc.compile()
    return nc


def _pack_bias450(v):
    vp = _pad470(np.asarray(v, np.float32).reshape(450, 1))[:, 0]
    arr = np.zeros((128, 6), np.float32)
    arr[:, 0], arr[:, 1], arr[:, 2] = vp[0:128], vp[128:256], vp[256:384]
    arr[0:22, 3] = vp[384:406]   # u rem
    arr[0:22, 4] = vp[416:438]   # i rem
    arr[0:22, 5] = vp[448:470]   # o rem
    return arr


def _pack_bias150(v):
    v = np.asarray(v, np.float32)
    arr = np.zeros((128, 2), np.float32)
    arr[:, 0] = v[0:128]
    arr[0:22, 1] = v[128:150]
    return arr


# test-harness hooks: set _RUN_KWARGS["trace"]=True before calling kernel() to
# capture an NTFF profile; the BassKernelResults lands in _LAST[0].
_RUN_KWARGS = {}
_LAST = [None]


def kernel(x, W_ioux, b_ioux, W_iouh, b_iouh, W_fx, b_fx, W_fh, b_fh):
    # W_fx/b_fx only affect leaf forget gates, which multiply zero child cells
    # in the reference; they do not influence the output.
    nc = _build()
    xp = np.asarray(x, np.float32)[:, BREV, :]
    consts = {
        "wx": np.ascontiguousarray(_pad470(np.asarray(W_ioux, np.float32)).T),
        "wh": np.ascontiguousarray(_pad470(np.asarray(W_iouh, np.float32)).T),
        "wf": np.ascontiguousarray(np.asarray(W_fh, np.float32).T),
        "bl": _pack_bias450(np.asarray(b_ioux) + np.asarray(b_iouh)),
        "bi": _pack_bias450(b_iouh),
        "bf": _pack_bias150(b_fh),
        "idd": np.eye(128, dtype=np.float32),
    }
    in_maps = []
    for c in range(NCORES):
        xc = xp[c * TC:(c + 1) * TC]                          # [128, 256, 300]
        xb = xc.reshape(NB, B, L, IN).transpose(0, 2, 1, 3)   # [16, 256, 8, 300]
        in_maps.append({"x": np.ascontiguousarray(xb.reshape(NB, CB, IN)), **consts})
    res = run_bass_kernel_spmd(nc, in_maps, core_ids=list(range(NCORES)),
                               **_RUN_KWARGS)
    _LAST[0] = res
    return np.concatenate([res.results[c]["out"] for c in range(NCORES)], axis=1)
